# revision 2
# baseline (speedup 1.0000x reference)
"""DeepseekV3 decoder layer on 8 Trainium2 NeuronCores (Bass/Tile) — v2.

Sharding as baseline: token shards (256/core) for q_a/kv_a/o_proj/router/
shared-expert, head shards (2/core) for MLA attention, expert shards
(1/core) for routed experts.

v2 changes vs baseline:
- latents are rms-normalized BEFORE the AllGather and sent in bf16
  (halves ag1 bytes, removes all stat-row plumbing)
- router-weight AllGather merged into the h AllGather (4 collectives total)
- most weights bf16; fp32 kept for the router chain and residual adds
- large-N matmuls (512/1024 free dim) for PE p-state + fewer instructions
- consolidated DMAs; weight prefetch overlaps collectives; shared expert
  overlaps the h AllGather
"""
import sys
NOCOLL = False

if "/opt/trn_rl_repo" not in sys.path:
    sys.path.insert(0, "/opt/trn_rl_repo")

import numpy as np
import ml_dtypes

import concourse.bass as bass
import concourse.bacc as bacc
import concourse.tile as tile
from concourse import mybir
from concourse import bass_utils

FP = mybir.dt.float32
BF = mybir.dt.bfloat16
FR = mybir.dt.float32r
AF = mybir.ActivationFunctionType
ALU = mybir.AluOpType

# flash attention score-operand dtype (q/k). FR is safer numerically.
# probs/v (the av side) are always bf16.
FLASH_BF = False
FD = BF if FLASH_BF else FR

NCORE = 8
B, S, H = 2, 1024, 2048
T = B * S
NH, DN, DR, DV = 16, 128, 64, 128
DQK = DN + DR
KVL, QL = 512, 1536
NE, NG, INTER = 8, 4, 768
TSH = T // NCORE          # 256
HPC = NH // NCORE         # 2
SCALING = float(DQK) ** -0.5
RSF = 2.5
EPS = 1e-6

AG1_ROWS = QL + KVL + DR       # 2112
AG2_ROWS = H + NE              # 2056


def fr(ap):
    return ap.bitcast(FR)


def build_program():
    nc = bacc.Bacc("TRN2", target_bir_lowering=False, debug=False,
                   num_devices=NCORE)

    def din(name, shape, dtype=FP):
        return nc.dram_tensor(name, shape, dtype, kind="ExternalInput").ap()

    hidT = din("hidT", [H, TSH])
    qa_wT = din("qa_wT", [H, QL], BF)
    kva_wT = din("kva_wT", [H, KVL + DR], BF)
    qb_wT = din("qb_wT", [QL, HPC * DQK], BF)   # cols: h0n h1n h0A h0B h1A h1B
    kvb_wT = din("kvb_wT", [KVL, HPC * 256], BF)  # cols: k0 k1 v0 v1
    o_wT = din("o_wT", [NH * DV, H], BF)
    r_wT = din("r_wT", [H, NE])
    r_bias = din("r_bias", [NE, 1])
    onehot = din("onehot", [NE, 1], BF)
    g_wT = din("g_wT", [H, INTER], BF)
    u_wT = din("u_wT", [H, INTER], BF)
    d_wT = din("d_wT", [INTER, H], BF)
    sg_wT = din("sg_wT", [H, INTER], BF)
    su_wT = din("su_wT", [H, INTER], BF)
    sd_wT = din("sd_wT", [INTER, H], BF)
    cc_q = din("cc_q", [128, T])
    ss_q = din("ss_q", [128, T])
    cc_k = din("cc_k", [DR, TSH])
    ss_k = din("ss_k", [DR, TSH])
    maskT_d = din("maskT", [512, 512], BF)
    Gm_d = din("Gm", [NE, NG])
    Dg_d = din("Dg", [NG, NG * NG])
    Rg_d = din("Rg", [NG * NG, NG])
    Em_d = din("Em", [NG, NE])
    De_d = din("De", [NE, NE * NE])
    Re_d = din("Re", [NE * NE, NE])

    out = nc.dram_tensor("out", [H, TSH], FP, kind="ExternalOutput").ap()

    RG = [list(range(NCORE))]

    def dma(out_ap, in_ap):
        nc.sync.dma_start(out_ap, in_ap)

    def kp(ap, p=128):
        return ap.rearrange("(k p) t -> p k t", p=p)

    tcx = tile.TileContext(nc)
    tc = tcx.__enter__()
    dram_cm = tc.tile_pool(name="dram", bufs=1, space="DRAM")
    dram = dram_cm.__enter__()
    pp_cm = tc.tile_pool(name="persist", bufs=1)
    pp = pp_cm.__enter__()

    ag1_in = dram.tile([AG1_ROWS, TSH], BF)
    ag1_out = dram.tile([NCORE * AG1_ROWS, TSH], BF, **({} if NOCOLL else dict(addr_space="Shared")))
    a2a_in = dram.tile([NCORE * 256, TSH], BF)
    a2a_out = dram.tile([NCORE * 256, TSH], BF)
    ag2_in = dram.tile([AG2_ROWS, TSH], BF)
    ag2_out = dram.tile([NCORE * AG2_ROWS, TSH], BF, **({} if NOCOLL else dict(addr_space="Shared")))
    rs_in = dram.tile([NCORE * H, TSH], BF)
    rs_out = dram.tile([H, TSH], BF)

    ones_fr = pp.tile([128, 1], FP)
    nc.vector.memset(ones_fr[:], 1.0)
    ones_bf = pp.tile([128, 1], BF)
    nc.vector.memset(ones_bf[:], 1.0)
    epsb = pp.tile([128, 1], FP)
    nc.vector.memset(epsb[:], EPS)

    x2s = pp.tile([128, 16, TSH], FP)
    hb = pp.tile([128, 16, TSH], BF)

    ag1v = ag1_out.rearrange("(j r) t -> j r t", r=AG1_ROWS)
    ag2v = ag2_out.rearrange("(j r) t -> j r t", r=AG2_ROWS)

    # ==================== phase A: local normalized latents ===============
    with tc.tile_pool(name="pA", bufs=1) as pa, \
         tc.tile_pool(name="pAt", bufs=2) as pat, \
         tc.tile_pool(name="psA", bufs=2, space="PSUM") as psa:

        x0b = pa.tile([128, 16, TSH], BF)
        nc.gpsimd.dma_start(x0b[:], kp(hidT))   # cast f32->bf16 in DMA
        qaw = pa.tile([128, 16, QL], BF)
        dma(qaw[:], kp(qa_wT))
        kvaw = pa.tile([128, 16, KVL + DR], BF)
        dma(kvaw[:], kp(kva_wT))
        cck = pa.tile([DR, TSH], FP)
        dma(cck[:], cc_k[:])
        ssk = pa.tile([DR, TSH], FP)
        dma(ssk[:], ss_k[:])

        # rstd of x (stats off bf16 copy; fine: scale cancels in q/kv norms)
        ss_ps = psa.tile([1, TSH], FP, tag="st")
        for k in range(16):
            sq = pat.tile([128, TSH], FR, tag="sq")
            nc.scalar.square(sq[:], x0b[:, k, :])
            nc.tensor.matmul(ss_ps[:], fr(ones_fr[:]), fr(sq[:]),
                             start=(k == 0), stop=(k == 15))
        rstd1 = pa.tile([1, TSH], FP)
        nc.scalar.activation(rstd1[:], ss_ps[:], AF.Sqrt,
                             bias=epsb[0:1, :], scale=1.0 / H)
        nc.vector.reciprocal(rstd1[:], rstd1[:])
        rsq1 = pa.tile([1, TSH], FP)
        nc.scalar.square(rsq1[:], rstd1[:])

        # q_a raw -> stats -> fold (rstd1*r2) in one multiply, bf16 out
        qa_s = pa.tile([128, 12, TSH], FP)
        ss2 = psa.tile([1, TSH], FP, tag="st")
        for m in range(12):
            ps = psa.tile([128, TSH], FP, tag="mm")
            for k in range(16):
                nc.tensor.matmul(ps[:], qaw[:, k, 128 * m:128 * (m + 1)],
                                 x0b[:, k, :],
                                 start=(k == 0), stop=(k == 15))
            sq = pat.tile([128, TSH], FR, tag="sq")
            nc.scalar.square(sq[:], ps[:])
            nc.tensor.matmul(ss2[:], fr(ones_fr[:]), fr(sq[:]),
                             start=(m == 0), stop=(m == 11),
                             skip_group_check=True)
            nc.scalar.copy(qa_s[:, m, :], ps[:])
        t2 = pa.tile([1, TSH], FP)
        nc.vector.tensor_mul(t2[:], ss2[:], rsq1[:])
        r2 = pa.tile([1, TSH], FP)
        nc.scalar.activation(r2[:], t2[:], AF.Sqrt,
                             bias=epsb[0:1, :], scale=1.0 / QL)
        nc.vector.reciprocal(r2[:], r2[:])
        nc.vector.tensor_mul(r2[:], r2[:], rstd1[:])
        b2 = pa.tile([128, TSH], FP)
        nc.gpsimd.partition_broadcast(b2[:], r2[:1, :])
        qa_n = pa.tile([128, 12, TSH], BF)
        for m in range(12):
            nc.vector.tensor_mul(qa_n[:, m, :], qa_s[:, m, :], b2[:])

        # kv_a raw -> stats -> fold; krot gets rstd1 only
        ckv_s = pa.tile([128, 4, TSH], FP)
        kr_raw = pa.tile([64, TSH], FP)
        ss3 = psa.tile([1, TSH], FP, tag="st")
        for m in range(5):
            mc = 128 if m < 4 else 64
            ps = psa.tile([128, TSH], FP, tag="mm")
            for k in range(16):
                nc.tensor.matmul(ps[:mc, :],
                                 kvaw[:, k, 128 * m:128 * m + mc],
                                 x0b[:, k, :],
                                 start=(k == 0), stop=(k == 15))
            if m < 4:
                sq = pat.tile([128, TSH], FR, tag="sq")
                nc.scalar.square(sq[:], ps[:])
                nc.tensor.matmul(ss3[:], fr(ones_fr[:]), fr(sq[:]),
                                 start=(m == 0), stop=(m == 3),
                                 skip_group_check=True)
                nc.scalar.copy(ckv_s[:, m, :], ps[:])
            else:
                nc.scalar.copy(kr_raw[:], ps[:64, :])
        t3 = pa.tile([1, TSH], FP)
        nc.vector.tensor_mul(t3[:], ss3[:], rsq1[:])
        r3 = pa.tile([1, TSH], FP)
        nc.scalar.activation(r3[:], t3[:], AF.Sqrt,
                             bias=epsb[0:1, :], scale=1.0 / KVL)
        nc.vector.reciprocal(r3[:], r3[:])
        nc.vector.tensor_mul(r3[:], r3[:], rstd1[:])
        b3 = pa.tile([128, TSH], FP)
        nc.gpsimd.partition_broadcast(b3[:], r3[:1, :])
        ckv_n = pa.tile([128, 4, TSH], BF)
        for m in range(4):
            nc.vector.tensor_mul(ckv_n[:, m, :], ckv_s[:, m, :], b3[:])

        # local k rope (rot rows pre-permuted to [A(32) B(32)] on host)
        b1 = pa.tile([128, TSH], FP)
        nc.gpsimd.partition_broadcast(b1[:], rstd1[:1, :])
        kr_sh = pa.tile([64, TSH], FP)
        dma(kr_sh[0:32, :], kr_raw[32:64, :])
        dma(kr_sh[32:64, :], kr_raw[0:32, :])
        nc.vector.tensor_mul(kr_sh[:], kr_sh[:], ssk[:])
        krf = pa.tile([64, TSH], FP)
        nc.vector.tensor_mul(krf[:], kr_raw[:], cck[:])
        nc.vector.tensor_add(krf[:], krf[:], kr_sh[:])
        kr = pa.tile([64, TSH], BF)
        nc.vector.tensor_mul(kr[:], krf[:], b1[:64, :])

        dma(ag1_in[0:QL, :].rearrange("(m p) t -> p m t", p=128), qa_n[:])
        dma(ag1_in[QL:QL + KVL, :].rearrange("(m p) t -> p m t", p=128),
            ckv_n[:])
        dma(ag1_in[QL + KVL:AG1_ROWS, :], kr[:])

    if NOCOLL:
        for _j in range(NCORE):
            nc.sync.dma_start(
                ag1_out[AG1_ROWS * _j:AG1_ROWS * (_j + 1), :], ag1_in[:, :])
    else:
        nc.gpsimd.collective_compute(
            "AllGather", ALU.bypass, replica_groups=RG,
            ins=[ag1_in.opt()], outs=[ag1_out.opt()])

    # ==================== attention (2 heads/core, all T) =================
    att_cm = tc.tile_pool(name="att", bufs=1)
    at = att_cm.__enter__()
    atp_cm = tc.tile_pool(name="atp", bufs=2)
    atp = atp_cm.__enter__()
    pst_cm = tc.tile_pool(name="psT", bufs=2, space="PSUM")
    pst = pst_cm.__enter__()

    qb_sb = at.tile([128, 12, HPC * DQK], BF)
    dma(qb_sb[:], kp(qb_wT))
    kvb_sb = at.tile([128, 4, HPC * 256], BF)
    dma(kvb_sb[:], kp(kvb_wT))
    maskT = at.tile([128, 4, 512], BF)
    dma(maskT[:], kp(maskT_d))

    qn = at.tile([128, 2, T], FD)
    qr = at.tile([128, T], FD)
    qr1 = at.tile([64, T], FD)
    kn = at.tile([128, 2, T], FD)
    krotg = at.tile([64, T], FD)
    vt = at.tile([128, 16, TSH], BF)
    attn = at.tile([128, 2, T], BF)

    with tc.tile_pool(name="proj", bufs=1) as pj, \
         tc.tile_pool(name="projs", bufs=2) as pjs:

        if FLASH_BF:
            dma(krotg[:].rearrange("r (j t) -> r j t", t=TSH),
                ag1v[:, QL + KVL:AG1_ROWS, :].rearrange("j r t -> r j t"))
        else:
            krbf = pj.tile([64, T], BF)
            dma(krbf[:].rearrange("r (j t) -> r j t", t=TSH),
                ag1v[:, QL + KVL:AG1_ROWS, :].rearrange("j r t -> r j t"))
            nc.scalar.copy(krotg[:], krbf[:])

        for n4 in range(4):
            nsl = slice(512 * n4, 512 * (n4 + 1))
            qrhs = pjs.tile([128, 12, 512], BF, tag="qrhs", bufs=1)
            for jj in range(2):
                j = 2 * n4 + jj
                dma(qrhs[:, :, 256 * jj:256 * (jj + 1)],
                    ag1v[j, 0:QL, :].rearrange("(k p) t -> p k t", p=128))
            lat = pjs.tile([128, 4, 512], BF, tag="lat")
            for jj in range(2):
                j = 2 * n4 + jj
                dma(lat[:, :, 256 * jj:256 * (jj + 1)],
                    ag1v[j, QL:QL + KVL, :].rearrange("(k p) t -> p k t",
                                                      p=128))
            qro = pjs.tile([128, 512], FP, tag="qro")
            for m in range(3):
                ps = pst.tile([128, 512], FP, tag="mm")
                for k in range(12):
                    nc.tensor.matmul(
                        ps[:], qb_sb[:, k, 128 * m:128 * (m + 1)],
                        qrhs[:, k, :], start=(k == 0), stop=(k == 11))
                if m < 2:
                    nc.scalar.copy(qn[:, m, nsl], ps[:])
                else:
                    nc.scalar.copy(qro[:], ps[:])
            # rope this chunk
            qsh = pjs.tile([128, 512], FP, tag="qsh")
            dma(qsh[0:32, :], qro[32:64, :])
            dma(qsh[32:64, :], qro[0:32, :])
            dma(qsh[64:96, :], qro[96:128, :])
            dma(qsh[96:128, :], qro[64:96, :])
            ccn = pjs.tile([128, 512], FP, tag="ccn")
            dma(ccn[:], cc_q[:, nsl])
            ssn = pjs.tile([128, 512], FP, tag="ssn")
            dma(ssn[:], ss_q[:, nsl])
            nc.vector.tensor_mul(qsh[:], qsh[:], ssn[:])
            nc.vector.tensor_mul(qro[:], qro[:], ccn[:])
            nc.vector.tensor_add(qro[:], qro[:], qsh[:])
            nc.scalar.copy(qr[:, nsl], qro[:])

            # k_nope for this chunk (both heads)
            for h in range(2):
                ps = pst.tile([128, 512], FP, tag="mm")
                for k in range(4):
                    nc.tensor.matmul(
                        ps[:], kvb_sb[:, k, 128 * h:128 * (h + 1)],
                        lat[:, k, :], start=(k == 0), stop=(k == 3))
                nc.scalar.copy(kn[:, h, nsl], ps[:])

            # v transposed: [tok_p, vcols] per 128-token slice
            for s2 in range(4):
                ps = pst.tile([128, 256], FP, tag="mm")
                for k in range(4):
                    nc.tensor.matmul(
                        ps[:], lat[:, k, 128 * s2:128 * (s2 + 1)],
                        kvb_sb[:, k, 256:512], start=(k == 0), stop=(k == 3))
                nc.scalar.copy(vt[:, 4 * n4 + s2, :], ps[:])

        dma(qr1[:], qr[64:128, :])

    # flash attention, scores transposed [k_p, q_f]
    for b_ in range(2):
        for h in range(2):
            for sqi in range(2):
                q0 = 1024 * b_ + 512 * sqi
                qsl = slice(q0, q0 + 512)
                nk = 4 * (sqi + 1)
                aps = pst.tile([128, 512], FP, tag="av")
                dps = pst.tile([1, 512], FP, tag="dn")
                for sk in range(nk):
                    k0 = 1024 * b_ + 128 * sk
                    ksl = slice(k0, k0 + 128)
                    sps = pst.tile([128, 512], FP, tag="sc")
                    nc.tensor.matmul(sps[:], kn[:, h, ksl], qn[:, h, qsl],
                                     start=True, stop=False)
                    qrh = qr[0:64, qsl] if h == 0 else qr1[:, qsl]
                    nc.tensor.matmul(sps[:], krotg[:, ksl], qrh,
                                     start=False, stop=True)
                    pr = atp.tile([128, 512], BF, tag="pr", bufs=2)
                    nc.scalar.activation(pr[:], sps[:], AF.Exp,
                                         scale=SCALING)
                    if sk >= 4 * sqi:
                        nc.vector.tensor_mul(
                            pr[:], pr[:], maskT[:, sk - 4 * sqi, :])
                    nc.tensor.matmul(
                        aps[:], vt[:, 8 * b_ + sk, 128 * h:128 * (h + 1)],
                        pr[:], start=(sk == 0), stop=(sk == nk - 1),
                        skip_group_check=True)
                    nc.tensor.matmul(
                        dps[:], ones_bf[:], pr[:],
                        start=(sk == 0), stop=(sk == nk - 1),
                        skip_group_check=True)
                rd = atp.tile([1, 512], FP, tag="rd", bufs=1)
                nc.vector.reciprocal(rd[:], dps[:])
                rdb = atp.tile([128, 512], FP, tag="rdb", bufs=1)
                nc.gpsimd.partition_broadcast(rdb[:], rd[:1, :])
                nc.vector.tensor_mul(attn[:, h, qsl], aps[:], rdb[:])

    a2av = a2a_in.rearrange("(j h p) t -> j p h t", h=2, p=128)
    for j in range(NCORE):
        dma(a2av[j], attn[:, :, TSH * j:TSH * (j + 1)])

    if NOCOLL:
        nc.sync.dma_start(a2a_out[:, :], a2a_in[:, :])
    else:
        nc.gpsimd.collective_compute(
            "AllToAll", ALU.bypass, replica_groups=RG,
            ins=[a2a_in.opt()], outs=[a2a_out.opt()])

    pst_cm.__exit__(None, None, None)
    atp_cm.__exit__(None, None, None)
    att_cm.__exit__(None, None, None)

    # ==================== o_proj + ln2 + router ====================
    with tc.tile_pool(name="op", bufs=1) as pop, \
         tc.tile_pool(name="opt", bufs=2) as pot, \
         tc.tile_pool(name="psO", bufs=2, space="PSUM") as pso:

        o_sb = pop.tile([128, 16, H], BF)
        dma(o_sb[:], kp(o_wT))
        x0r = pop.tile([128, 16, TSH], FP)
        dma(x0r[:], kp(hidT))
        hs = pop.tile([128, 16, TSH], FP)
        attn_sb = pop.tile([128, 16, TSH], BF)
        dma(attn_sb[:], kp(a2a_out[:, :]))

        for m in range(16):
            ps = pso.tile([128, TSH], FP, tag="mm")
            for k in range(16):
                nc.tensor.matmul(ps[:], o_sb[:, k, 128 * m:128 * (m + 1)],
                                 attn_sb[:, k, :],
                                 start=(k == 0), stop=(k == 15))
            nc.vector.tensor_add(x2s[:, m, :], ps[:], x0r[:, m, :])

        ss4 = pso.tile([1, TSH], FP, tag="st")
        for k in range(16):
            sq = pot.tile([128, TSH], FR, tag="sq")
            nc.scalar.square(sq[:], x2s[:, k, :])
            nc.tensor.matmul(ss4[:], fr(ones_fr[:]), fr(sq[:]),
                             start=(k == 0), stop=(k == 15))
        r4 = pop.tile([1, TSH], FP)
        nc.scalar.activation(r4[:], ss4[:], AF.Sqrt,
                             bias=epsb[0:1, :], scale=1.0 / H)
        nc.vector.reciprocal(r4[:], r4[:])
        b4 = pop.tile([128, TSH], FP)
        nc.gpsimd.partition_broadcast(b4[:], r4[:1, :])
        for m in range(16):
            nc.vector.tensor_mul(hs[:, m, :], x2s[:, m, :], b4[:])
            nc.vector.tensor_mul(hb[:, m, :], x2s[:, m, :], b4[:])
        dma(ag2_in[0:H, :].rearrange("(m p) t -> p m t", p=128), hb[:])

        # router (fp32 matmuls, baseline-proven chain)
        rw_sb = pop.tile([128, 16, NE], FP)
        dma(rw_sb[:], kp(r_wT))
        rb_sb = pop.tile([NE, 1], FP)
        dma(rb_sb[:], r_bias[:])
        Gm_s = pop.tile([NE, NG], FP)
        dma(Gm_s[:], Gm_d[:])
        Dg_s = pop.tile([NG, 16], FP)
        dma(Dg_s[:], Dg_d[:])
        Rg_s = pop.tile([16, NG], FP)
        dma(Rg_s[:], Rg_d[:])
        Em_s = pop.tile([NG, NE], FP)
        dma(Em_s[:], Em_d[:])
        De_s = pop.tile([NE, 64], FP)
        dma(De_s[:], De_d[:])
        Re_s = pop.tile([64, NE], FP)
        dma(Re_s[:], Re_d[:])

        lg = pso.tile([NE, TSH], FP, tag="rt")
        for k in range(16):
            nc.tensor.matmul(lg[:], rw_sb[:, k, :], hs[:, k, :],
                             start=(k == 0), stop=(k == 15))
        sr = pop.tile([NE, TSH], FP)
        nc.scalar.activation(sr[:], lg[:], AF.Sigmoid)
        sc_t = pop.tile([NE, TSH], FP)
        nc.vector.tensor_scalar(sc_t[:], sr[:], rb_sb[:, 0:1], None, ALU.add)
        gs_ps = pso.tile([NG, TSH], FP, tag="rt")
        nc.tensor.matmul(gs_ps[:], Gm_s[:], sc_t[:])
        gs_sb = pop.tile([NG, TSH], FP)
        nc.scalar.copy(gs_sb[:], gs_ps[:])
        gd_ps = pso.tile([16, TSH], FP, tag="rt")
        nc.tensor.matmul(gd_ps[:], Dg_s[:], gs_sb[:])
        gp = pop.tile([16, TSH], FP)
        nc.vector.tensor_scalar(gp[:], gd_ps[:], 0.0, None, ALU.is_gt)
        gc_ps = pso.tile([NG, TSH], FP, tag="rt")
        nc.tensor.matmul(gc_ps[:], Rg_s[:], gp[:])
        gm = pop.tile([NG, TSH], FP)
        nc.vector.tensor_scalar(gm[:], gc_ps[:], 2.0, None, ALU.is_lt)
        em_ps = pso.tile([NE, TSH], FP, tag="rt")
        nc.tensor.matmul(em_ps[:], Em_s[:], gm[:])
        msk = pop.tile([NE, TSH], FP)
        nc.vector.tensor_mul(msk[:], em_ps[:], sc_t[:])
        ed_ps = pso.tile([64, TSH], FP, tag="rt")
        nc.tensor.matmul(ed_ps[:], De_s[:], msk[:])
        ep = pop.tile([64, TSH], FP)
        nc.vector.tensor_scalar(ep[:], ed_ps[:], 0.0, None, ALU.is_gt)
        ec_ps = pso.tile([NE, TSH], FP, tag="rt")
        nc.tensor.matmul(ec_ps[:], Re_s[:], ep[:])
        es = pop.tile([NE, TSH], FP)
        nc.vector.tensor_scalar(es[:], ec_ps[:], 2.0, None, ALU.is_lt)
        w_sb = pop.tile([NE, TSH], FP)
        nc.vector.tensor_mul(w_sb[:], es[:], sr[:])
        ws_ps = pso.tile([1, TSH], FP, tag="rt")
        nc.tensor.matmul(ws_ps[:], ones_fr[0:NE, :], w_sb[:])
        wse = pop.tile([1, TSH], FP)
        nc.vector.tensor_scalar(wse[:], ws_ps[:], 1e-20, None, ALU.add)
        nc.vector.reciprocal(wse[:], wse[:])
        wb = pop.tile([NE, TSH], FP)
        nc.gpsimd.partition_broadcast(wb[:], wse[:1, :])
        dw_sb = pop.tile([NE, TSH], BF)
        nc.vector.scalar_tensor_tensor(dw_sb[:], w_sb[:], RSF, wb[:],
                                       ALU.mult, ALU.mult)
        dma(ag2_in[H:AG2_ROWS, :], dw_sb[:])

    # expert weights load here: overlaps ag2 + shared expert
    pm_cm = tc.tile_pool(name="moe", bufs=1)
    pm = pm_cm.__enter__()
    gw_sb = pm.tile([128, 16, INTER], BF)
    dma(gw_sb[:], kp(g_wT))
    uw_sb = pm.tile([128, 16, INTER], BF)
    dma(uw_sb[:], kp(u_wT))

    if NOCOLL:
        for _j in range(NCORE):
            nc.sync.dma_start(
                ag2_out[AG2_ROWS * _j:AG2_ROWS * (_j + 1), :], ag2_in[:, :])
    else:
        nc.gpsimd.collective_compute(
            "AllGather", ALU.bypass, replica_groups=RG,
            ins=[ag2_in.opt()], outs=[ag2_out.opt()])

    # ============== shared expert (overlaps the h AllGather) ==============
    with tc.tile_pool(name="sh", bufs=1) as psh, \
         tc.tile_pool(name="sht", bufs=2) as psht, \
         tc.tile_pool(name="psS", bufs=2, space="PSUM") as pss:

        sg_sb = psh.tile([128, 16, INTER], BF)
        dma(sg_sb[:], kp(sg_wT))
        su_sb = psh.tile([128, 16, INTER], BF)
        dma(su_sb[:], kp(su_wT))
        sd_sb = psh.tile([128, 6, H], BF)
        dma(sd_sb[:], kp(sd_wT))

        act2 = psh.tile([128, 6, TSH], BF)
        for m in range(6):
            g2 = pss.tile([128, TSH], FP, tag="mg")
            for k in range(16):
                nc.tensor.matmul(g2[:], sg_sb[:, k, 128 * m:128 * (m + 1)],
                                 hb[:, k, :], start=(k == 0), stop=(k == 15))
            g2s = psht.tile([128, TSH], FP, tag="g2s")
            nc.scalar.activation(g2s[:], g2[:], AF.Sigmoid)
            nc.vector.tensor_mul(g2s[:], g2[:], g2s[:])
            u2 = pss.tile([128, TSH], FP, tag="mg")
            for k in range(16):
                nc.tensor.matmul(u2[:], su_sb[:, k, 128 * m:128 * (m + 1)],
                                 hb[:, k, :], start=(k == 0), stop=(k == 15))
            nc.vector.tensor_mul(act2[:, m, :], u2[:], g2s[:])
        for m in range(16):
            d2 = pss.tile([128, TSH], FP, tag="md")
            for k in range(6):
                nc.tensor.matmul(d2[:], sd_sb[:, k, 128 * m:128 * (m + 1)],
                                 act2[:, k, :], start=(k == 0), stop=(k == 5))
            nc.vector.tensor_add(x2s[:, m, :], d2[:], x2s[:, m, :])

    # ==================== routed expert (bf16, all T) ====================
    dwn_sb = pm.tile([128, 6, H], BF)
    dma(dwn_sb[:], kp(d_wT))
    with tc.tile_pool(name="moet", bufs=2) as pmt, \
         tc.tile_pool(name="psM", bufs=2, space="PSUM") as psm:

        # routing weight row for my expert -> [128, T] broadcast
        oh_sb = pm.tile([NE, 1], BF)
        dma(oh_sb[:], onehot[:])
        dwg = pm.tile([NE, NCORE, TSH], BF)
        dma(dwg[:], ag2v[:, H:AG2_ROWS, :].rearrange("j r t -> r j t"))
        bcf = pm.tile([1, T], BF)
        for q in range(4):
            ewp = psm.tile([1, 512], FP, tag="rt", bufs=1)
            nc.tensor.matmul(ewp[:], oh_sb[:],
                             dwg[:].rearrange(
                                 "r j t -> r (j t)")[:, 512 * q:512 * (q + 1)])
            nc.scalar.copy(bcf[:, 512 * q:512 * (q + 1)], ewp[:])
        bce = pm.tile([128, T], BF)
        nc.gpsimd.partition_broadcast(bce[:], bcf[:1, :])

        rsv = rs_in.rearrange("(j m p) t -> p m j t", m=16, p=128)
        for c in range(4):
            csl = slice(512 * c, 512 * (c + 1))
            hbn = pmt.tile([128, 16, 512], BF, tag="hb")
            for jj in range(2):
                j = 2 * c + jj
                dma(hbn[:, :, 256 * jj:256 * (jj + 1)],
                    ag2v[j, 0:H, :].rearrange("(k p) t -> p k t", p=128))
            act_n = pmt.tile([128, 6, 512], BF, tag="act", bufs=1)
            for m in range(6):
                gp_ = psm.tile([128, 512], FP, tag="mg")
                for k in range(16):
                    nc.tensor.matmul(gp_[:],
                                     gw_sb[:, k, 128 * m:128 * (m + 1)],
                                     hbn[:, k, :],
                                     start=(k == 0), stop=(k == 15))
                gsi = pmt.tile([128, 512], FP, tag="gsi")
                nc.scalar.activation(gsi[:], gp_[:], AF.Sigmoid)
                nc.vector.tensor_mul(gsi[:], gp_[:], gsi[:])
                up_ = psm.tile([128, 512], FP, tag="mg")
                for k in range(16):
                    nc.tensor.matmul(up_[:],
                                     uw_sb[:, k, 128 * m:128 * (m + 1)],
                                     hbn[:, k, :],
                                     start=(k == 0), stop=(k == 15))
                nc.vector.tensor_mul(act_n[:, m, :], up_[:], gsi[:])
                nc.vector.tensor_mul(act_n[:, m, :], act_n[:, m, :],
                                     bce[:, csl])
            for m in range(16):
                dp = psm.tile([128, 512], FP, tag="md")
                for k in range(6):
                    nc.tensor.matmul(dp[:],
                                     dwn_sb[:, k, 128 * m:128 * (m + 1)],
                                     act_n[:, k, :],
                                     start=(k == 0), stop=(k == 5))
                eo = pmt.tile([128, 512], BF, tag="eo", bufs=2)
                nc.scalar.copy(eo[:], dp[:])
                dma(rsv[:, m, 2 * c:2 * c + 2, :],
                    eo[:].rearrange("p (j t) -> p j t", t=TSH))

        if NOCOLL:
            nc.sync.dma_start(rs_out[:, :], rs_in[0:H, :])
        else:
            nc.gpsimd.collective_compute(
                "ReduceScatter", ALU.add, replica_groups=RG,
                ins=[rs_in.opt()], outs=[rs_out.opt()])

        rsb = pm.tile([128, 16, TSH], BF)
        dma(rsb[:], kp(rs_out[:, :]))
        for m in range(16):
            nc.vector.tensor_add(x2s[:, m, :], rsb[:, m, :], x2s[:, m, :])
        dma(out[:, :].rearrange("(m p) t -> p m t", p=128), x2s[:])

    pm_cm.__exit__(None, None, None)
    pp_cm.__exit__(None, None, None)
    dram_cm.__exit__(None, None, None)
    tcx.__exit__(None, None, None)

    nc.compile()
    return nc


# --------------------------------------------------------------------------
# host side
# --------------------------------------------------------------------------

_PERM64 = np.concatenate([np.arange(0, 64, 2), np.arange(1, 64, 2)])


def _routing_mats():
    Gm = np.zeros((NE, NG), np.float32)
    for g in range(NG):
        Gm[2 * g, g] = 1.0
        Gm[2 * g + 1, g] = 1.0
    Dg = np.zeros((NG, NG * NG), np.float32)
    Rg = np.zeros((NG * NG, NG), np.float32)
    for i in range(NG):
        for j in range(NG):
            p = i * NG + j
            Dg[i, p] += 1.0
            Dg[j, p] -= 1.0
            Rg[p, j] = 1.0
    Em = np.zeros((NG, NE), np.float32)
    for g in range(NG):
        Em[g, 2 * g] = 1.0
        Em[g, 2 * g + 1] = 1.0
    De = np.zeros((NE, NE * NE), np.float32)
    Re = np.zeros((NE * NE, NE), np.float32)
    for i in range(NE):
        for j in range(NE):
            p = i * NE + j
            De[i, p] += 1.0
            De[j, p] -= 1.0
            Re[p, j] = 1.0
    return Gm, Dg, Rg, Em, De, Re


def _c(a):
    return np.ascontiguousarray(a, dtype=np.float32)


def _bfc(a):
    return np.ascontiguousarray(np.asarray(a, np.float32).astype(
        ml_dtypes.bfloat16))


def make_in_maps(inputs):
    f32 = np.float32
    hs_ = np.asarray(inputs["hidden_states"], f32).reshape(T, H)
    cos = np.asarray(inputs["cos"], f32).reshape(T, DR)
    sin = np.asarray(inputs["sin"], f32).reshape(T, DR)
    ln1 = np.asarray(inputs["ln1_w"], f32)
    ln2 = np.asarray(inputs["ln2_w"], f32)
    qaln = np.asarray(inputs["q_a_ln_w"], f32)
    kvln = np.asarray(inputs["kv_a_ln_w"], f32)

    qa_w = np.asarray(inputs["q_a_w"], f32) * ln1[None, :]
    kva_w = np.asarray(inputs["kv_a_w"], f32) * ln1[None, :]
    kva_w = np.concatenate([kva_w[:KVL], kva_w[KVL:][_PERM64]], 0)
    qb_w = np.asarray(inputs["q_b_w"], f32) * qaln[None, :]
    kvb_w = np.asarray(inputs["kv_b_w"], f32) * kvln[None, :]
    o_w = np.asarray(inputs["o_w"], f32)
    r_w = np.asarray(inputs["router_w"], f32) * ln2[None, :]
    r_b = np.asarray(inputs["router_bias"], f32)
    g_w = np.asarray(inputs["gate_w"], f32) * ln2[None, None, :]
    u_w = np.asarray(inputs["up_w"], f32) * ln2[None, None, :]
    d_w = np.asarray(inputs["down_w"], f32)
    sg_w = np.asarray(inputs["sh_gate_w"], f32) * ln2[None, :]
    su_w = np.asarray(inputs["sh_up_w"], f32) * ln2[None, :]
    sd_w = np.asarray(inputs["sh_down_w"], f32)

    cosT = cos.T
    sinT = sin.T
    cc_q = np.concatenate([cosT[0:32], cosT[32:64]] * 2, 0)
    ss_q = np.concatenate([-sinT[0:32], sinT[32:64]] * 2, 0)
    maskT = np.triu(np.ones((512, 512), np.float32))
    Gm, Dg, Rg, Em, De, Re = _routing_mats()

    shared = dict(
        qa_wT=_bfc(qa_w.T), kva_wT=_bfc(kva_w.T), o_wT=_bfc(o_w.T),
        r_wT=_c(r_w.T), r_bias=_c(r_b.reshape(NE, 1)),
        sg_wT=_bfc(sg_w.T), su_wT=_bfc(su_w.T), sd_wT=_bfc(sd_w.T),
        cc_q=_c(cc_q), ss_q=_c(ss_q), maskT=_bfc(maskT),
        Gm=_c(Gm), Dg=_c(Dg), Rg=_c(Rg), Em=_c(Em), De=_c(De), Re=_c(Re),
    )

    in_maps = []
    for c in range(NCORE):
        tsl = slice(TSH * c, TSH * (c + 1))
        h0, h1 = 2 * c, 2 * c + 1
        qb_cols = [qb_w[DQK * h0:DQK * h0 + DN],
                   qb_w[DQK * h1:DQK * h1 + DN]]
        for h in (h0, h1):
            rot = qb_w[DQK * h + DN:DQK * (h + 1)]
            qb_cols.append(rot[0::2])
            qb_cols.append(rot[1::2])
        qb_c = np.concatenate(qb_cols, 0)              # [384, QL]
        kvb_c = np.concatenate(
            [kvb_w[256 * h0:256 * h0 + 128],
             kvb_w[256 * h1:256 * h1 + 128],
             kvb_w[256 * h0 + 128:256 * h0 + 256],
             kvb_w[256 * h1 + 128:256 * h1 + 256]], 0)  # [512, KVL]
        oh = np.zeros((NE, 1), np.float32)
        oh[c, 0] = 1.0
        m = dict(shared)
        m.update(
            hidT=_c(hs_[tsl].T),
            qb_wT=_bfc(qb_c.T), kvb_wT=_bfc(kvb_c.T),
            cc_k=_c(cosT[:, tsl]),
            ss_k=_c(np.concatenate([-sinT[0:32, tsl],
                                    sinT[32:64, tsl]], 0)),
            onehot=_bfc(oh),
            g_wT=_bfc(g_w[c].T), u_wT=_bfc(u_w[c].T), d_wT=_bfc(d_w[c].T),
        )
        in_maps.append(m)
    return in_maps


_NC_CACHE = None


def _get_nc():
    global _NC_CACHE
    if _NC_CACHE is None:
        _NC_CACHE = build_program()
    return _NC_CACHE


def kernel(**inputs) -> np.ndarray:
    nc = _get_nc()
    in_maps = make_in_maps(inputs)
    res = bass_utils.run_bass_kernel_spmd(nc, in_maps,
                                          core_ids=list(range(NCORE)))
    full = np.empty((H, T), np.float32)
    for c in range(NCORE):
        full[:, TSH * c:TSH * (c + 1)] = res.results[c]["out"]
    return np.ascontiguousarray(full.T).reshape(B, S, H)
